# revision 1
# baseline (speedup 1.0000x reference)
"""Trainium2 Bass kernel for a ternary-weight ResNet BasicBlock.

Reference computation (all fp32):
    out = htanh(BN2(conv3x3(htanh(BN1(conv3x3(x, tern(w1)))), tern(w2)) + x))
with training-mode BN (global batch stats over (N, H, W)).

Strategy (per core, 4 of 32 images):
  - channels (64) on partitions; two images share the 128-partition dim with
    block-diagonal duplicated weights -> each matmul convolves two images.
  - conv3x3 = 9 accumulating matmuls over shifted views of a padded bf16
    plane in SBUF.  The residual is added during PSUM evacuation (DVE).
  - BN stats: ACT-evac accum_out (sums) + DVE square accum (sum of squares),
    PE-transpose cross-half fold, one (sum, sumsq) AllReduce across 8 cores.
  - BN1 affine folds into conv2: clamp(v, lo_c, hi_c) with per-channel
    bounds, conv2 weights pre-scaled by s1, pad ring set to -b1/s1, and the
    constant bias term (sum_w2 @ b1) folded into BN2's statistics/affine.
"""

import numpy as np
import ml_dtypes

import concourse.bacc as bacc
import concourse.bass as bass
from concourse import mybir
from concourse import tile
from concourse import bass_utils

F32 = mybir.dt.float32
BF16 = mybir.dt.bfloat16
ALU = mybir.AluOpType
ACTF = mybir.ActivationFunctionType

# Problem constants (hardcoded per contract)
N, C, HH, WW = 32, 64, 112, 112
NCORES = 8
NPC = N // NCORES          # images per core (4)
SLOTS = 2                  # image slots per partition half
DELTA = 0.3
EPS = 1e-5

P = 128
WP = WW + 2                # padded cols (114)
HP = HH + 3                # storage rows: 1 guard + 114 padded (115)
PLANE = HP * WP            # 13110
XBF = SLOTS * PLANE + 8    # flat free size w/ tail guard (26228)
RPT = 4                    # output rows per PSUM tile
NT = HH // RPT             # 28 row tiles
NFREE = RPT * WP           # matmul moving free size (456)
NP_PART = float(SLOTS * HH * WW)  # elements per partition per conv output
BLK = 16                   # output rows per staged DMA block

TAPS = [(ky - 1, kx - 1) for ky in range(3) for kx in range(3)]


def _stats_allreduce(nc, tag, sp, dp, psp1, st, eye128, eye2, groups, no_cc):
    """st [128,2] per-partition (sum, sumsq) -> gst [128,2] global per-channel
    totals (both halves identical).  Cross-half fold and the partition
    broadcast are done with PE transposes; one DRAM AllReduce round-trip."""
    psT = psp1.tile([2, P], F32, name=f"psT{tag}", tag="stats")
    stT = sp.tile([2, P], F32, name=f"stT{tag}")
    gstT = sp.tile([2, P], F32, name=f"gstT{tag}")
    bin_ = dp.tile([2, 64], F32, name=f"bin{tag}")
    bout = dp.tile([2, 64], F32, name=f"bout{tag}")
    psB = psp1.tile([P, 8], F32, name=f"psB{tag}", tag="stats")
    gst = sp.tile([P, 2], F32, name=f"gst{tag}")
    nc.tensor.transpose(psT[:], st[:], eye128[:])
    nc.scalar.activation(stT[:], psT[:], ACTF.Copy)
    nc.vector.scalar_tensor_tensor(stT[:, 0:64], stT[:, 0:64], 1.0,
                                   stT[:, 64:128], ALU.mult, ALU.add)
    nc.sync.dma_start(bin_[:], stT[:, 0:64])
    if no_cc:
        nc.sync.dma_start(bout[:], bin_[:])
    else:
        nc.gpsimd.collective_compute(
            "AllReduce", ALU.add, replica_groups=groups,
            ins=[bin_.opt()], outs=[bout.opt()])
    nc.sync.dma_start(gstT[:, 0:64], bout[:])
    nc.vector.tensor_copy(gstT[:, 64:128], gstT[:, 0:64])
    nc.tensor.transpose(psB[:, 0:2], gstT[:], eye2[:])
    nc.scalar.activation(gst[:], psB[:, 0:2], ACTF.Copy)
    return gst


def _bn_scale_bias(nc, name, gst, gamma, beta, pool, n_total):
    """From global (sum, sumsq) [128,2] compute per-partition scale/bias
    [128,1] implementing x -> (x - mean) * rsqrt(var + eps) * gamma + beta."""
    mex = pool.tile([P, 2], F32, name=f"{name}_mex")
    mean = mex[:, 0:1]
    ex2 = mex[:, 1:2]
    msq = pool.tile([P, 1], F32, name=f"{name}_msq")
    var = pool.tile([P, 1], F32, name=f"{name}_var")
    std = pool.tile([P, 1], F32, name=f"{name}_std")
    rstd = pool.tile([P, 1], F32, name=f"{name}_rstd")
    seff = pool.tile([P, 1], F32, name=f"{name}_seff")
    nms = pool.tile([P, 1], F32, name=f"{name}_nms")
    beff = pool.tile([P, 1], F32, name=f"{name}_beff")
    inv_n = 1.0 / n_total
    nc.vector.tensor_scalar(mex[:], gst[:], inv_n, None, ALU.mult)
    nc.vector.scalar_tensor_tensor(msq[:], mean, 1.0, mean, ALU.mult, ALU.mult)
    nc.vector.scalar_tensor_tensor(var[:], ex2, 1.0, msq[:], ALU.mult, ALU.subtract)
    nc.vector.tensor_scalar(var[:], var[:], EPS, None, ALU.add)
    nc.scalar.activation(std[:], var[:], ACTF.Sqrt, bias=0.0, scale=1.0)
    nc.vector.reciprocal(rstd[:], std[:])
    nc.vector.scalar_tensor_tensor(seff[:], rstd[:], 1.0, gamma[:], ALU.mult, ALU.mult)
    nc.vector.scalar_tensor_tensor(nms[:], mean, -1.0, seff[:], ALU.mult, ALU.mult)
    nc.vector.scalar_tensor_tensor(beff[:], nms[:], 1.0, beta[:], ALU.mult, ALU.add)
    return seff, beff


def build_nc(repeat=1, num_devices=NCORES, no_cc=False):
    nc = bacc.Bacc("TRN2", target_bir_lowering=False, debug=False,
                   num_devices=num_devices)

    xa = nc.dram_tensor("xa", (P, SLOTS, HH, WW), BF16, kind="ExternalInput")
    w1s = nc.dram_tensor("w1s", (P, 9 * P), BF16, kind="ExternalInput")
    w2s = nc.dram_tensor("w2s", (P, 9 * P), BF16, kind="ExternalInput")
    w2sum = nc.dram_tensor("w2sum", (P, P), BF16, kind="ExternalInput")
    eye128d = nc.dram_tensor("eye128", (P, P), F32, kind="ExternalInput")
    eye2d = nc.dram_tensor("eye2", (2, 2), F32, kind="ExternalInput")
    g1e = nc.dram_tensor("g1e", (P, 1), F32, kind="ExternalInput")
    b1e = nc.dram_tensor("b1e", (P, 1), F32, kind="ExternalInput")
    g2e = nc.dram_tensor("g2e", (P, 1), F32, kind="ExternalInput")
    b2e = nc.dram_tensor("b2e", (P, 1), F32, kind="ExternalInput")
    outd = nc.dram_tensor("out", (P, SLOTS, HH, WW), BF16, kind="ExternalOutput")

    groups = [list(range(num_devices))]
    n_total = float(num_devices * NPC * HH * WW)

    with tile.TileContext(nc) as tc:
        with (
            tc.tile_pool(name="persist", bufs=1) as pp,
            tc.tile_pool(name="psum", bufs=6, space="PSUM") as psp,
            tc.tile_pool(name="psum1", bufs=1, space="PSUM") as psp1,
            tc.tile_pool(name="sqsp", bufs=3) as sqp,
            tc.tile_pool(name="stage", bufs=4) as stp,
            tc.tile_pool(name="dram", bufs=1, space="DRAM") as dp,
            tc.tile_pool(name="small", bufs=1) as sp,
        ):
            # ---- persistent SBUF buffers ----
            xb = pp.tile([P, XBF], BF16, name="xb")
            act = pp.tile([P, XBF], BF16, name="act")
            o2 = pp.tile([P, SLOTS * HH * WW], BF16, name="o2")
            w1t = pp.tile([P, 9 * P], BF16, name="w1t")
            w2t = pp.tile([P, 9 * P], BF16, name="w2t")
            w2x = pp.tile([P, 9 * P], BF16, name="w2x")   # s1-scaled conv2 taps
            w2sm = pp.tile([P, P], BF16, name="w2sm")
            eye128 = pp.tile([P, P], F32, name="eye128t")
            eye2 = pp.tile([2, 2], F32, name="eye2t")
            g1t = pp.tile([P, 1], F32, name="g1t")
            b1t = pp.tile([P, 1], F32, name="b1t")
            g2t = pp.tile([P, 1], F32, name="g2t")
            b2t = pp.tile([P, 1], F32, name="b2t")
            # per-(rowtile, slot) partial sums / sums of squares
            s1p = pp.tile([P, NT * SLOTS], F32, name="s1p")
            q1p = pp.tile([P, NT * SLOTS], F32, name="q1p")
            s2p = pp.tile([P, NT * SLOTS], F32, name="s2p")
            q2p = pp.tile([P, NT * SLOTS], F32, name="q2p")

            xb4 = xb[:, 0:SLOTS * PLANE].rearrange(
                "p (s r c) -> p s r c", s=SLOTS, r=HP, c=WP)
            act4 = act[:, 0:SLOTS * PLANE].rearrange(
                "p (s r c) -> p s r c", s=SLOTS, r=HP, c=WP)
            o24 = o2[:].rearrange("p (s r c) -> p s r c", s=SLOTS, r=HH, c=WW)

            # ---- zero pad borders (x: conv1 needs zero pad; act ring is
            # rewritten per-iteration but must be defined for the RMW) ----
            for s in range(SLOTS):
                for buf4 in (xb4, act4):
                    nc.gpsimd.memset(buf4[:, s, 0:2, :], 0.0)
                    nc.gpsimd.memset(buf4[:, s, HP - 1:HP, :], 0.0)
                    nc.gpsimd.memset(buf4[:, s, 2:HP - 1, 0:1], 0.0)
                    nc.gpsimd.memset(buf4[:, s, 2:HP - 1, WP - 1:WP], 0.0)
            nc.gpsimd.memset(xb[:, SLOTS * PLANE:XBF], 0.0)
            nc.gpsimd.memset(act[:, SLOTS * PLANE:XBF], 0.0)

            # ---- conv1-critical loads first: w1, then x row chunks ----
            nc.sync.dma_start(w1t[:], w1s[:])
            # warm the PE pstate during the x-load with tiny dummy matmuls
            for i in range(24):
                psw = psp1.tile([P, 8], F32, name="psw", tag="stats")
                nc.tensor.matmul(psw[0:8, 0:8], w1t[:, 0:8], w1t[:, 8:16])
            DROWS = 16
            for j in range(HH // DROWS):
                r0 = DROWS * j
                for s in range(SLOTS):
                    nc.sync.dma_start(
                        xb4[:, s, 2 + r0:2 + r0 + DROWS, 1:1 + WW],
                        xa[:, s, r0:r0 + DROWS, :])
            nc.sync.dma_start(w2t[:], w2s[:])
            nc.sync.dma_start(w2sm[:], w2sum[:])
            nc.sync.dma_start(eye128[:], eye128d[:])
            nc.sync.dma_start(eye2[:], eye2d[:])
            nc.sync.dma_start(g1t[:], g1e[:])
            nc.sync.dma_start(b1t[:], b1e[:])
            nc.sync.dma_start(g2t[:], g2e[:])
            nc.sync.dma_start(b2t[:], b2e[:])

            for _ in range(repeat):
                # ================= phase A: conv1 =================
                for rt in range(NT):
                    py0 = RPT * rt + 1
                    for s in range(SLOTS):
                        ps = psp.tile([P, RPT * WW], F32, name="ps")
                        for t, (dy, dx) in enumerate(TAPS):
                            r0 = py0 + dy + 1
                            nc.tensor.matmul(
                                ps[:], w1t[:, t * P:(t + 1) * P],
                                xb4[:, s, r0:r0 + RPT, 1 + dx:1 + dx + WW],
                                start=(t == 0), stop=(t == 8))
                        ps_int = ps[:].rearrange("p (r c) -> p r c", r=RPT, c=WW)
                        idx = rt * SLOTS + s
                        # evacuate raw conv1 (pre-BN) into act interior
                        dst = act4[:, s, py0 + 1:py0 + 1 + RPT, 1:1 + WW]
                        nc.scalar.activation(dst, ps_int, ACTF.Copy,
                                             accum_out=s1p[:, idx:idx + 1])
                        sqs = sqp.tile([P, RPT, WW], BF16, name="sqs")
                        nc.vector.scalar_tensor_tensor(
                            sqs[:], dst, 1.0, dst, ALU.mult, ALU.mult,
                            accum_out=q1p[:, idx:idx + 1])

                # ---- stats 1: reduce, all-reduce, derive fold params ----
                st1 = sp.tile([P, 2], F32, name="st1")
                nc.vector.tensor_reduce(st1[:, 0:1], s1p[:],
                                        mybir.AxisListType.X, ALU.add)
                nc.vector.tensor_reduce(st1[:, 1:2], q1p[:],
                                        mybir.AxisListType.X, ALU.add)
                gst1 = _stats_allreduce(nc, "1", sp, dp, psp1, st1,
                                        eye128, eye2, groups, no_cc)
                s1e, bb1 = _bn_scale_bias(nc, "bn1", gst1, g1t, b1t, sp,
                                          n_total)
                # clamp bounds: a=(-1-b1)/s1, b=(1-b1)/s1; lo=min, hi=max;
                # pad value cpad = -b1/s1  (so s1*cpad + b1 == 0)
                invs = sp.tile([P, 1], F32, name="invs")
                ca = sp.tile([P, 1], F32, name="ca")
                cb = sp.tile([P, 1], F32, name="cb")
                lo1 = sp.tile([P, 1], F32, name="lo1")
                hi1 = sp.tile([P, 1], F32, name="hi1")
                cpad = sp.tile([P, 1], F32, name="cpad")
                b1f = sp.tile([P, 1], BF16, name="b1f")
                nc.vector.reciprocal(invs[:], s1e[:])
                nc.vector.tensor_scalar(ca[:], bb1[:], 1.0, -1.0, ALU.add,
                                        ALU.mult)           # -(1+b1)
                nc.vector.scalar_tensor_tensor(ca[:], ca[:], 1.0, invs[:],
                                               ALU.mult, ALU.mult)
                nc.vector.tensor_scalar(cb[:], bb1[:], -1.0, 1.0, ALU.mult,
                                        ALU.add)            # (1-b1)
                nc.vector.scalar_tensor_tensor(cb[:], cb[:], 1.0, invs[:],
                                               ALU.mult, ALU.mult)
                nc.vector.scalar_tensor_tensor(lo1[:], ca[:], 1.0, cb[:],
                                               ALU.mult, ALU.min)
                nc.vector.scalar_tensor_tensor(hi1[:], ca[:], 1.0, cb[:],
                                               ALU.mult, ALU.max)
                nc.vector.scalar_tensor_tensor(cpad[:], bb1[:], -1.0, invs[:],
                                               ALU.mult, ALU.mult)
                # set act pad ring to cpad (gpsimd: off the DVE critical path)
                for s in range(SLOTS):
                    for ring in (act4[:, s, 1:2, :], act4[:, s, HP - 1:HP, :],
                                 act4[:, s, 2:HP - 1, 0:1],
                                 act4[:, s, 2:HP - 1, WP - 1:WP]):
                        nc.gpsimd.tensor_scalar(ring, ring, 0.0, cpad[:],
                                                ALU.mult, ALU.add)

                # ====== phase B: clamp (in place) interleaved w/ conv2 ======
                def clamp_chunk(j):
                    r0 = RPT * j + 2
                    for s in range(SLOTS):
                        ch = act4[:, s, r0:r0 + RPT, 1:1 + WW]
                        nc.vector.tensor_scalar(ch, ch, lo1[:], hi1[:],
                                                ALU.max, ALU.min)

                clamp_chunk(0)
                clamp_chunk(1)
                # scale conv2 taps by s1 (per input channel = partition)
                for t in range(9):
                    nc.vector.tensor_scalar(w2x[:, t * P:(t + 1) * P],
                                            w2t[:, t * P:(t + 1) * P],
                                            s1e[:], None, ALU.mult)
                for rt in range(NT):
                    if rt + 2 < NT:
                        clamp_chunk(rt + 2)
                    py0 = RPT * rt + 1
                    for s in range(SLOTS):
                        ps = psp.tile([P, RPT * WW], F32, name="ps")
                        for t, (dy, dx) in enumerate(TAPS):
                            r0 = py0 + dy + 1
                            nc.tensor.matmul(
                                ps[:], w2x[:, t * P:(t + 1) * P],
                                act4[:, s, r0:r0 + RPT, 1 + dx:1 + dx + WW],
                                start=(t == 0), stop=(t == 8))
                        ps_int = ps[:].rearrange("p (r c) -> p r c", r=RPT, c=WW)
                        idx = rt * SLOTS + s
                        dst = o24[:, s, RPT * rt:RPT * rt + RPT, :]
                        xres = xb4[:, s, py0 + 1:py0 + 1 + RPT, 1:1 + WW]
                        # evacuate + residual add (+ per-channel sum)
                        nc.vector.scalar_tensor_tensor(
                            dst, ps_int, 1.0, xres, ALU.mult, ALU.add,
                            accum_out=s2p[:, idx:idx + 1])
                        sqs = sqp.tile([P, RPT, WW], BF16, name="sqs")
                        nc.vector.scalar_tensor_tensor(
                            sqs[:], dst, 1.0, dst, ALU.mult, ALU.mult,
                            accum_out=q2p[:, idx:idx + 1])

                # bias2[m] = sum_k w2sum[k,m] * b1[k]
                nc.vector.tensor_copy(b1f[:], bb1[:])
                psb = psp1.tile([P, 8], F32, name="psb", tag="stats")
                nc.tensor.matmul(psb[:, 0:1], w2sm[:], b1f[:])
                bias2 = sp.tile([P, 1], F32, name="bias2")
                nc.scalar.activation(bias2[:], psb[:, 0:1], ACTF.Copy)

                # ---- stats 2 (o2 excludes bias2; correct the moments) ----
                st2 = sp.tile([P, 2], F32, name="st2")
                u1 = sp.tile([P, 1], F32, name="u1")
                u2 = sp.tile([P, 1], F32, name="u2")
                nc.vector.tensor_reduce(st2[:, 0:1], s2p[:],
                                        mybir.AxisListType.X, ALU.add)
                nc.vector.tensor_reduce(st2[:, 1:2], q2p[:],
                                        mybir.AxisListType.X, ALU.add)
                # qadj = q + 2*bias2*sum' + np*bias2^2 ; sadj = sum' + np*bias2
                nc.vector.scalar_tensor_tensor(u1[:], bias2[:], 2.0,
                                               st2[:, 0:1], ALU.mult, ALU.mult)
                nc.vector.scalar_tensor_tensor(u2[:], bias2[:], NP_PART,
                                               bias2[:], ALU.mult, ALU.mult)
                nc.vector.scalar_tensor_tensor(u1[:], u1[:], 1.0, u2[:],
                                               ALU.mult, ALU.add)
                nc.vector.scalar_tensor_tensor(st2[:, 1:2], st2[:, 1:2], 1.0,
                                               u1[:], ALU.mult, ALU.add)
                nc.vector.scalar_tensor_tensor(st2[:, 0:1], bias2[:], NP_PART,
                                               st2[:, 0:1], ALU.mult, ALU.add)
                gst2 = _stats_allreduce(nc, "2", sp, dp, psp1, st2,
                                        eye128, eye2, groups, no_cc)
                s2e, bb2 = _bn_scale_bias(nc, "bn2", gst2, g2t, b2t, sp,
                                          n_total)
                # o2 lacks bias2: bb2' = bb2 + bias2*s2
                bb2f = sp.tile([P, 1], F32, name="bb2f")
                nc.vector.scalar_tensor_tensor(bb2f[:], bias2[:], 1.0, s2e[:],
                                               ALU.mult, ALU.mult)
                nc.vector.scalar_tensor_tensor(bb2f[:], bb2f[:], 1.0, bb2[:],
                                               ALU.mult, ALU.add)

                # ========= phase C: affine+htanh -> fp32 out =========
                for b in range(HH // BLK):
                    for s in range(SLOTS):
                        stg = stp.tile([P, BLK, WW], BF16, name="stg")
                        for j in range(BLK // RPT):
                            r0 = BLK * b + RPT * j
                            sub = stg[:, RPT * j:RPT * j + RPT, :]
                            nc.vector.tensor_scalar(
                                sub, o24[:, s, r0:r0 + RPT, :],
                                s2e[:], bb2f[:], ALU.mult, ALU.add)
                            nc.vector.tensor_scalar(sub, sub, -1.0, 1.0,
                                                    ALU.max, ALU.min)
                        nc.sync.dma_start(
                            outd[:, s, BLK * b:BLK * b + BLK, :], stg[:])

    nc.compile()
    return nc


def _prep_weights(w):
    """w (64,64,3,3) fp32 -> ternarized block-diag stationaries
    [128, 9*128] bf16 where tap t stationary [k, m] = W[m, k, ky, kx]."""
    q = (np.sign(w) * (np.abs(w) > DELTA)).astype(np.float32)
    wt = q.transpose(2, 3, 1, 0).reshape(9, C, C)  # [t, k(cin), m(cout)]
    out = np.zeros((P, 9, P), np.float32)
    out[0:C, :, 0:C] = wt.transpose(1, 0, 2)
    out[C:P, :, C:P] = wt.transpose(1, 0, 2)
    return out.reshape(P, 9 * P).astype(ml_dtypes.bfloat16)


def _prep_w2sum(w):
    """Block-diag sum over taps of ternarized w2: [128, 128] bf16."""
    q = (np.sign(w) * (np.abs(w) > DELTA)).astype(np.float32)
    ws = q.sum(axis=(2, 3)).T  # [k(cin), m(cout)]
    out = np.zeros((P, P), np.float32)
    out[0:C, 0:C] = ws
    out[C:P, C:P] = ws
    return out.astype(ml_dtypes.bfloat16)


def _shard_x(x):
    """x (32,64,112,112) fp32 -> per-core [128, 2, 112, 112] bf16 arrays."""
    shards = []
    for c in range(NCORES):
        xs = x[c * NPC:(c + 1) * NPC]  # (4,64,112,112)
        xbv = xs.reshape(2, SLOTS, C, HH, WW).transpose(0, 2, 1, 3, 4)
        xbv = np.ascontiguousarray(xbv.reshape(P, SLOTS, HH, WW))
        shards.append(xbv.astype(ml_dtypes.bfloat16))
    return shards


_NC_CACHE = {}


def _get_nc(repeat=1):
    if repeat not in _NC_CACHE:
        _NC_CACHE[repeat] = build_nc(repeat=repeat)
    return _NC_CACHE[repeat]


def make_in_maps(x, w1, g1, b1, w2, g2, b2):
    w1sv = _prep_weights(np.asarray(w1))
    w2sv = _prep_weights(np.asarray(w2))
    w2su = _prep_w2sum(np.asarray(w2))
    eye = np.eye(P, dtype=np.float32)

    def expand(v):
        return np.ascontiguousarray(
            np.tile(np.asarray(v, np.float32), 2)[:, None])

    shards = _shard_x(np.asarray(x, np.float32))
    return [{
        "xa": shards[c],
        "w1s": w1sv, "w2s": w2sv, "w2sum": w2su, "eye128": eye,
        "eye2": np.eye(2, dtype=np.float32),
        "g1e": expand(g1), "b1e": expand(b1),
        "g2e": expand(g2), "b2e": expand(b2),
    } for c in range(NCORES)]


def unshard_out(results):
    outs = []
    for c in range(NCORES):
        o = np.asarray(results[c]["out"]).astype(np.float32)
        o = o.reshape(2, C, SLOTS, HH, WW).transpose(0, 2, 1, 3, 4)
        outs.append(o.reshape(NPC, C, HH, WW))
    return np.concatenate(outs, axis=0)


def run(x, w1, g1, b1, w2, g2, b2, repeat=1):
    nc = _get_nc(repeat)
    in_maps = make_in_maps(x, w1, g1, b1, w2, g2, b2)
    res = bass_utils.run_bass_kernel_spmd(nc, in_maps,
                                          core_ids=list(range(NCORES)))
    return unshard_out(res.results)


def kernel(x, w1, g1, b1, w2, g2, b2):
    return run(x, w1, g1, b1, w2, g2, b2, repeat=1)

